# revision 12
# baseline (speedup 1.0000x reference)
"""Trainium2 Bass kernel for nn_AudioTokenPredictor (12-layer dense transformer).

Strategy: 8-way sequence-parallel over (batch, token-chunk) pairs.  Each core
owns 256 tokens: chunk i of batch 0 and chunk 7-i of batch 1 (balances causal
attention work).  Weights are replicated; per layer one AllGather shares K/V
(bf16) across cores.  All activations are kept feature-major (x^T) so every
GEMM and the whole attention (scores computed kv-major as s^T) need no
on-device transposes.  GEMMs run fp32 via the float32r full-rate path (or
bf16, configurable); attention internals (q,k,v,p) are bf16 with fp32
accumulation.  Embedding lookup and the tiny final head run on host in fp32.

Self-contained: all shapes/sharding hardcoded.
"""

import functools
import numpy as np

# model dims (hardcoded from the problem spec)
L, H, D, DH, FF, CTX, S, V = 12, 12, 768, 64, 3072, 1536, 8, 1024
B, T = 2, 1024
NCORES = 8
CH = 128            # tokens per chunk
TOK = 2 * CH        # tokens owned per core
KD = D // 128       # 6 k-tiles over D
FFK = FF // 128     # 24 tiles over FF
MQK = (2 * D) // 128  # 12 M-tiles for the fused q,k GEMM
EPS = 1e-6
HG = 4              # heads per score group
NGRP = H // HG      # 3
SCALE = DH ** -0.5
CCN = 2 * D * TOK // 2  # elements (bf16) of one rank's K/V contribution: kT + v

# build configuration (flip for perf/accuracy tradeoffs)
W_BF16 = True      # weights in bf16 (halves weight DMA; needed to be compute-bound)
A_BF16 = True      # GEMM activations in bf16


# --------------------------------------------------------------------------
# device kernel
# --------------------------------------------------------------------------

def _build(n_layers=L, w_bf16=W_BF16, a_bf16=A_BF16, stage=99, asub=99, dbg=False, nr=NCORES):
    import concourse.bacc as bacc
    import concourse.tile as tile
    import concourse.mybir as mybir

    F32 = mybir.dt.float32
    F32R = mybir.dt.float32r
    BF16 = mybir.dt.bfloat16
    AF = mybir.ActivationFunctionType
    # float32r is the PE's reduced-precision fp32 fast path (full rate at
    # N>=256).  Values feeding an f32r matmul must be produced/rounded as f32r,
    # so GEMM-input tiles and weight tensors are declared float32r directly.
    wdt = BF16 if w_bf16 else F32R
    adt = BF16 if a_bf16 else F32R

    def R(ap):
        return ap

    nc = bacc.Bacc("TRN2", target_bir_lowering=False, debug=False,
                   num_devices=NCORES)

    xT0 = nc.dram_tensor("xT0", [D, TOK], F32, kind="ExternalInput")
    wqkvT = nc.dram_tensor("wqkvT", [n_layers, D, 3 * D], wdt, kind="ExternalInput")
    wprojT = nc.dram_tensor("wprojT", [n_layers, D, D], wdt, kind="ExternalInput")
    wgateT = nc.dram_tensor("wgateT", [n_layers, D, FF], wdt, kind="ExternalInput")
    wupT = nc.dram_tensor("wupT", [n_layers, D, FF], wdt, kind="ExternalInput")
    wdownT = nc.dram_tensor("wdownT", [n_layers, FF, D], wdt, kind="ExternalInput")
    masks = nc.dram_tensor("masks", [2, NCORES, CH, CH], BF16, kind="ExternalInput")
    xT_out = nc.dram_tensor("xT_out", [D, TOK], F32, kind="ExternalOutput")
    if dbg:
        d_qk = nc.dram_tensor("d_qk", [128, MQK, TOK], F32, kind="ExternalOutput")
        d_qodd = nc.dram_tensor("d_qodd", [64, KD, TOK], F32, kind="ExternalOutput")
        d_kT = nc.dram_tensor("d_kT", [64, H, CH], F32, kind="ExternalOutput")
        d_vp = nc.dram_tensor("d_vp", [128, H, DH + 1], F32, kind="ExternalOutput")
        d_pT = nc.dram_tensor("d_pT", [128, H, CH], F32, kind="ExternalOutput")
        d_oacc = nc.dram_tensor("d_oacc", [DH + 1, H * CH], F32, kind="ExternalOutput")

    xT0_t = xT0.rearrange("(a p) t -> p a t", p=128)       # [128, KD, TOK]
    xTo_t = xT_out.rearrange("(a p) t -> p a t", p=128)

    from contextlib import ExitStack
    with tile.TileContext(nc) as tc, ExitStack() as ctx:
        sb = ctx.enter_context(tc.tile_pool(name="sb", bufs=1))
        wpool = ctx.enter_context(tc.tile_pool(name="w", bufs=8))
        wdpool = ctx.enter_context(tc.tile_pool(name="wd", bufs=3))
        wvpool = ctx.enter_context(tc.tile_pool(name="wv", bufs=6))
        act = ctx.enter_context(tc.tile_pool(name="act", bufs=2))
        att = ctx.enter_context(tc.tile_pool(name="att", bufs=3))
        ffp = ctx.enter_context(tc.tile_pool(name="ff", bufs=3))
        pp = ctx.enter_context(tc.tile_pool(name="pp", bufs=1, space="PSUM"))
        dram = ctx.enter_context(tc.tile_pool(name="dram", bufs=2, space="DRAM"))

        # persistent tiles
        x = sb.tile([128, KD, TOK], F32, name="x_resid")
        qk_sb = sb.tile([128, MQK, TOK], BF16, name="qk_sb")
        q_odd = sb.tile([64, KD, TOK], BF16, name="q_odd")
        v_sb = sb.tile([128, 2, D], BF16, name="v_sb")
        o_allT = sb.tile([128, KD, TOK], adt, name="o_allT")
        ones_col = sb.tile([128, 1], adt, name="ones_col")
        eps_t = sb.tile([1, 1], F32, name="eps_t")
        nc.vector.memset(ones_col, 1.0)
        nc.vector.memset(eps_t, EPS)

        nc.sync.dma_start(x[:, :, :], xT0_t)

        def rmsnorm(xn):
            col = pp.tile([1, TOK], F32, tag="mm", bufs=2, name="colsum")
            for f in range(KD):
                x2 = act.tile([128, TOK], adt, tag="x2", bufs=3, name="x2")
                nc.vector.tensor_mul(x2, x[:, f, :], x[:, f, :])
                nc.tensor.matmul(col, lhsT=R(ones_col), rhs=R(x2),
                                 start=(f == 0), stop=(f == KD - 1))
            rstd = act.tile([1, TOK], F32, tag="rstd", bufs=2, name="rstd")
            nc.scalar.activation(out=rstd, in_=col, func=AF.Sqrt,
                                 bias=eps_t, scale=1.0 / D)
            rinv = act.tile([1, TOK], F32, tag="rinv", bufs=2, name="rinv")
            nc.vector.reciprocal(rinv, rstd)
            rbc = act.tile([128, TOK], F32, tag="rbc", bufs=2, name="rbc")
            nc.gpsimd.partition_broadcast(rbc, rinv)
            for f in range(KD):
                nc.vector.tensor_mul(xn[:, f, :], x[:, f, :], rbc)

        for l in range(n_layers):
            # ---------------- attention block ----------------
            xn = act.tile([128, KD, TOK], adt, tag="xn", bufs=2, name="xn")
            rmsnorm(xn)

            # fused q,k GEMM -> qk_sb (feature-major, bf16)
            for m in range(MQK):
                wt = wpool.tile([128, KD, 128], wdt, tag="w", bufs=8, name="w_qk")
                nc.sync.dma_start(
                    wt, wqkvT[l, :, 128 * m:128 * (m + 1)]
                    .rearrange("(a p) n -> p a n", p=128))
                ps = pp.tile([128, TOK], F32, tag="mm", bufs=2, name="qk_ps")
                for f in range(KD):
                    nc.tensor.matmul(ps, lhsT=R(wt[:, f, :]), rhs=R(xn[:, f, :]),
                                     start=(f == 0), stop=(f == KD - 1))
                nc.scalar.copy(out=qk_sb[:, m, :], in_=ps)
                if m < KD:
                    # odd head of this M-tile, shifted to base partition 0
                    nc.gpsimd.dma_start(q_odd[:, m, :], qk_sb[64:128, m, :])

            # v GEMM (token-major: v = xn^T.T @ Wv^T) -> v_sb bf16
            wv = []
            for f in range(KD):
                wvt = wvpool.tile([128, D], wdt, tag="wv", name="w_v")
                nc.sync.dma_start(wvt, wqkvT[l, 128 * f:128 * (f + 1), 2 * D:3 * D])
                wv.append(wvt)
            for m in range(2):
                for n in range(3):
                    ps = pp.tile([128, 256], F32, tag="mm", bufs=2, name="v_ps")
                    for f in range(KD):
                        nc.tensor.matmul(
                            ps, lhsT=R(xn[:, f, 128 * m:128 * (m + 1)]),
                            rhs=R(wv[f][:, 256 * n:256 * (n + 1)]),
                            start=(f == 0), stop=(f == KD - 1))
                    nc.vector.tensor_copy(
                        out=v_sb[:, m, 256 * n:256 * (n + 1)], in_=ps)

            if dbg and l == 0:
                dtmp = act.tile([128, MQK, TOK], F32, tag="dtmp", bufs=1, name="dtmp")
                nc.vector.tensor_copy(dtmp, qk_sb)
                nc.gpsimd.dma_start(d_qk[:, :, :], dtmp)
                dtmp2 = act.tile([64, KD, TOK], F32, tag="dtmp2", bufs=1, name="dtmp2")
                nc.vector.tensor_copy(dtmp2, q_odd)
                nc.gpsimd.dma_start(d_qodd[:, :, :], dtmp2)
            if stage <= 0:
                break
            # stage own K/V into the collective input, then AllGather
            cc_in = dram.tile([2 * D * TOK], BF16, tag="cc_in", bufs=2, name="cc_in")
            cc_out = dram.tile([NCORES, 2 * D * TOK], BF16, tag="cc_out", bufs=2,
                               addr_space="Shared", name="cc_out")
            kT_view = cc_in[0:D * TOK].rearrange("(f t) -> f t", t=TOK)
            v_view = cc_in[D * TOK:2 * D * TOK].rearrange("(t f) -> t f", f=D)
            for m in range(KD):
                nc.gpsimd.dma_start(kT_view[128 * m:128 * (m + 1), :],
                                    qk_sb[:, KD + m, :])
            for m in range(2):
                nc.gpsimd.dma_start(v_view[128 * m:128 * (m + 1), :], v_sb[:, m, :])
            nc.gpsimd.collective_compute(
                "AllGather", mybir.AluOpType.bypass,
                replica_groups=[list(range(NCORES))],
                ins=[cc_in.opt()], outs=[cc_out.opt()])

            if stage <= 1:
                break
            # attention: per own chunk c.  Phase A computes masked exp-scores
            # for every rank block (kept in SBUF); phase B accumulates each
            # head's AV over all ranks as one consecutive PSUM group (a PSUM
            # accumulation group must own its whole bank: start=True arms a
            # pending-zero for the full 2KB bank).
            for c in range(2):
                pTs, vps = [], []
                for r in range(nr):
                    rk = cc_out[r, 0:D * TOK].rearrange("(f t) -> f t", t=TOK)
                    rv = cc_out[r, D * TOK:2 * D * TOK].rearrange(
                        "(t f) -> t f", f=D)
                    kT = att.tile([64, H, CH], BF16, tag="kT", bufs=3, name="kT")
                    nc.gpsimd.dma_start(
                        kT, rk[:, CH * c:CH * (c + 1)]
                        .rearrange("(a p) t -> p a t", p=64))
                    vp = att.tile([128, H, DH + 1], BF16, tag="vp", bufs=8,
                                  name="vp")
                    nc.gpsimd.dma_start(
                        vp[:, :, 0:DH],
                        rv[CH * c:CH * (c + 1), :].rearrange("p (h d) -> p h d", h=H))
                    nc.vector.memset(vp[:, :, DH:DH + 1], 1.0)
                    mk = att.tile([CH, CH], BF16, tag="mk", bufs=3, name="mk")
                    nc.gpsimd.dma_start(mk, masks[c, r, :, :])
                    vps.append(vp)
                    if dbg and l == 0 and c == 0 and r == 0:
                        dk = att.tile([64, H, CH], F32, tag="dk", bufs=1, name="dk")
                        nc.vector.tensor_copy(dk, kT)
                        nc.gpsimd.dma_start(d_kT[:, :, :], dk)

                    pT = att.tile([128, H, CH], BF16, tag="pT", bufs=8, name="pT")
                    pTs.append(pT)
                    if asub <= 0:
                        continue
                    for g in range(NGRP):
                        sT = pp.tile([128, HG, CH], F32, tag="sT", bufs=2, name="sT")
                        for hh in range(HG):
                            h = g * HG + hh
                            if h % 2 == 0:
                                q_ap = qk_sb[0:64, h // 2, CH * c:CH * (c + 1)]
                            else:
                                q_ap = q_odd[:, h // 2, CH * c:CH * (c + 1)]
                            nc.tensor.matmul(
                                sT[:, hh, :],
                                lhsT=kT[:, h, :],
                                rhs=q_ap,
                                start=True, stop=True)
                        if asub <= 1:
                            continue
                        nc.scalar.activation(
                            out=pT[:, g * HG:(g + 1) * HG, :], in_=sT, func=AF.Exp)
                        if asub >= 3:
                            for hh in range(HG):
                                h = g * HG + hh
                                nc.vector.tensor_mul(pT[:, h, :], pT[:, h, :], mk)
                if dbg and l == 0 and c == 0:
                    dv = att.tile([128, H, DH + 1], F32, tag="dv", bufs=1, name="dv")
                    nc.vector.tensor_copy(dv, vps[0])
                    nc.gpsimd.dma_start(d_vp[:, :, :], dv)
                    dp = att.tile([128, H, CH], F32, tag="dp", bufs=1, name="dp")
                    nc.vector.tensor_copy(dp, pTs[0])
                    nc.gpsimd.dma_start(d_pT[:, :, :], dp)
                if asub <= 3:
                    continue
                # phase B: one bank-exclusive accumulation group per head
                for h in range(H):
                    oacc = pp.tile([DH + 1, CH], F32, tag="sT", bufs=2, name="oacc")
                    for r in range(nr):
                        nc.tensor.matmul(
                            oacc,
                            lhsT=vps[r][:, h, :], rhs=pTs[r][:, h, :],
                            start=(r == 0), stop=(r == nr - 1))
                    if dbg and l == 0 and c == 0:
                        do = act.tile([DH + 1, CH], F32, tag="do", bufs=2, name="do")
                        nc.vector.tensor_copy(do, oacc)
                        nc.gpsimd.dma_start(d_oacc[:, CH * h:CH * (h + 1)], do)
                    if asub < 5:
                        continue
                    linv = att.tile([1, CH], F32, tag="linv", bufs=2, name="linv")
                    nc.vector.reciprocal(linv, oacc[DH:DH + 1, :])
                    lbc = att.tile([DH, CH], F32, tag="lbc", bufs=2, name="lbc")
                    nc.gpsimd.partition_broadcast(lbc, linv)
                    bp = 64 * (h % 2)
                    nc.vector.tensor_mul(
                        o_allT[bp:bp + 64, h // 2, CH * c:CH * (c + 1)],
                        oacc[0:DH, :], lbc)

            # proj GEMM + residual
            for m in range(KD):
                wt = wpool.tile([128, KD, 128], wdt, tag="w", bufs=8, name="w_proj")
                nc.sync.dma_start(
                    wt, wprojT[l, :, 128 * m:128 * (m + 1)]
                    .rearrange("(a p) n -> p a n", p=128))
                ps = pp.tile([128, TOK], F32, tag="mm", bufs=2, name="proj_ps")
                for f in range(KD):
                    nc.tensor.matmul(ps, lhsT=R(wt[:, f, :]), rhs=R(o_allT[:, f, :]),
                                     start=(f == 0), stop=(f == KD - 1))
                nc.vector.tensor_add(x[:, m, :], x[:, m, :], ps)

            if stage <= 3:
                break
            # ---------------- FFN block ----------------
            xn2 = act.tile([128, KD, TOK], adt, tag="xn", bufs=2, name="xn2")
            rmsnorm(xn2)

            hks, wdks = [], []
            for k in range(FFK):
                wg = wpool.tile([128, KD, 128], wdt, tag="w", bufs=8, name="w_gate")
                nc.sync.dma_start(
                    wg, wgateT[l, :, 128 * k:128 * (k + 1)]
                    .rearrange("(a p) n -> p a n", p=128))
                gps = pp.tile([128, TOK], F32, tag="mm", bufs=2, name="g_ps")
                for f in range(KD):
                    nc.tensor.matmul(gps, lhsT=wg[:, f, :], rhs=xn2[:, f, :],
                                     start=(f == 0), stop=(f == KD - 1))
                g = ffp.tile([128, TOK], F32, tag="g", bufs=2, name="g_silu")
                nc.scalar.activation(out=g, in_=gps, func=AF.Silu)

                wu = wpool.tile([128, KD, 128], wdt, tag="w", bufs=8, name="w_up")
                nc.sync.dma_start(
                    wu, wupT[l, :, 128 * k:128 * (k + 1)]
                    .rearrange("(a p) n -> p a n", p=128))
                ups = pp.tile([128, TOK], F32, tag="mm", bufs=2, name="u_ps")
                for f in range(KD):
                    nc.tensor.matmul(ups, lhsT=wu[:, f, :], rhs=xn2[:, f, :],
                                     start=(f == 0), stop=(f == KD - 1))
                ht = ffp.tile([128, TOK], adt, tag="h", bufs=FFK, name="h_ff")
                nc.vector.tensor_mul(ht, ups, g)
                hks.append(ht)

                wdk = wdpool.tile([128, D], wdt, tag="wd", bufs=FFK, name="w_down")
                nc.sync.dma_start(wdk, wdownT[l, 128 * k:128 * (k + 1), :])
                wdks.append(wdk)
            for m in range(KD):
                dacc = pp.tile([128, TOK], F32, tag="dacc", bufs=2, name="dacc")
                for k in range(FFK):
                    nc.tensor.matmul(
                        dacc,
                        lhsT=wdks[k][:, 128 * m:128 * (m + 1)], rhs=hks[k],
                        start=(k == 0), stop=(k == FFK - 1))
                nc.vector.tensor_add(x[:, m, :], x[:, m, :], dacc)

        nc.sync.dma_start(xTo_t, x[:, :, :])

    nc.finalize()
    return nc


# --------------------------------------------------------------------------
# host-side: preprocessing, SPMD runner with replicated weights, postprocess
# --------------------------------------------------------------------------

_PER_CORE = ("xT0", "masks")


def _host_embed(inputs):
    rvq = np.asarray(inputs["rvq_embed"], dtype=np.float32)
    toks = np.asarray(inputs["prev_tokens"])
    pos = np.asarray(inputs["positions"])
    x = rvq[np.arange(S)[None, None, :], toks].sum(axis=2)
    x = x + np.asarray(inputs["pos_embed"], dtype=np.float32)[pos]
    x = x + np.asarray(inputs["intent_vec"], dtype=np.float32)
    return x  # [B, T, D] f32


def _prep_shared(inputs, n_layers, w_bf16):
    import ml_dtypes
    cast = (lambda a: a.astype(ml_dtypes.bfloat16)) if w_bf16 else \
           (lambda a: np.ascontiguousarray(a, dtype=np.float32))
    qkv = np.asarray(inputs["qkv_w"], dtype=np.float32).copy()
    n1 = np.asarray(inputs["norm1_w"], dtype=np.float32)
    n2 = np.asarray(inputs["norm2_w"], dtype=np.float32)
    qkv *= n1[:, None, :]
    qkv[:, 0:D, :] *= SCALE
    out = {
        "wqkvT": cast(np.transpose(qkv, (0, 2, 1))[:n_layers]),
        "wprojT": cast(np.transpose(
            np.asarray(inputs["proj_w"], dtype=np.float32), (0, 2, 1))[:n_layers]),
        "wgateT": cast(np.transpose(
            np.asarray(inputs["gate_w"], dtype=np.float32) * n2[:, None, :],
            (0, 2, 1))[:n_layers]),
        "wupT": cast(np.transpose(
            np.asarray(inputs["up_w"], dtype=np.float32) * n2[:, None, :],
            (0, 2, 1))[:n_layers]),
        "wdownT": cast(np.transpose(
            np.asarray(inputs["down_w"], dtype=np.float32), (0, 2, 1))[:n_layers]),
    }
    return out


def _prep_per_core(inputs):
    import ml_dtypes
    x0 = _host_embed(inputs)
    tri = np.triu(np.ones((CH, CH), np.float32))  # M[kv,q]=1 iff kv<=q
    ones = np.ones((CH, CH), np.float32)
    zeros = np.zeros((CH, CH), np.float32)
    per_core = []
    for i in range(NCORES):
        seg = np.concatenate(
            [x0[0, CH * i:CH * (i + 1)], x0[1, CH * (7 - i):CH * (8 - i)]], axis=0)
        xT0_i = np.ascontiguousarray(seg.T, dtype=np.float32)
        m = np.empty((2, NCORES, CH, CH), np.float32)
        for r in range(NCORES):
            m[0, r] = ones if r < i else (tri if r == i else zeros)
            m[1, r] = ones if r > i else (tri if r == i else zeros)
        per_core.append({"xT0": xT0_i, "masks": m.astype(ml_dtypes.bfloat16)})
    return per_core


def _postprocess(results, inputs):
    xb0 = results[7]["xT_out"][:, CH - 1]       # batch0 token 1023
    xb1 = results[0]["xT_out"][:, TOK - 1]      # batch1 token 1023
    xl = np.stack([xb0, xb1]).astype(np.float32)          # [2, D]
    rstd = 1.0 / np.sqrt((xl * xl).mean(axis=-1, keepdims=True) + EPS)
    xl = xl * rstd * np.asarray(inputs["normf_w"], dtype=np.float32)
    heads = np.asarray(inputs["heads_w"], dtype=np.float32)   # [S, V, D]
    logits = np.einsum("bd,svd->sbv", xl, heads)
    return logits.astype(np.float32)


class _Runner:
    """Compiled SPMD executable with replicated-weight sharding and cached
    on-device weights (rebuilt from bass2jax.run_bass_via_pjrt to avoid
    concatenating/transferring 8 copies of the weights)."""

    def __init__(self, nc):
        import jax
        import concourse.mybir as mybir
        from concourse import bass2jax
        from jax.sharding import Mesh, PartitionSpec, NamedSharding
        from jax.experimental.shard_map import shard_map

        bass2jax.install_neuronx_cc_hook()
        self._jax = jax
        self._bass2jax = bass2jax
        assert nc.dbg_addr is None
        partition_name = (nc.partition_id_tensor.name
                          if nc.partition_id_tensor else None)

        in_names, out_names, out_avals, zero_shapes = [], [], [], []
        for alloc in nc.m.functions[0].allocations:
            if not isinstance(alloc, mybir.MemoryLocationSet):
                continue
            name = alloc.memorylocations[0].name
            if alloc.kind == "ExternalInput":
                if name != partition_name:
                    in_names.append(name)
            elif alloc.kind == "ExternalOutput":
                shape = tuple(alloc.tensor_shape)
                dtype = mybir.dt.np(alloc.dtype)
                out_names.append(name)
                out_avals.append(jax.core.ShapedArray(shape, dtype))
                zero_shapes.append((shape, dtype))
        self.in_names = list(in_names)
        self.out_names = out_names
        self.out_avals = out_avals
        self.zero_shapes = zero_shapes
        n_params = len(in_names)
        n_outs = len(out_names)
        all_in = in_names + out_names
        if partition_name is not None:
            all_in.append(partition_name)

        donate = tuple(range(n_params, n_params + n_outs))

        def _body(*args):
            operands = list(args)
            if partition_name is not None:
                operands.append(bass2jax.partition_id_tensor())
            outs = bass2jax._bass_exec_p.bind(
                *operands,
                out_avals=tuple(out_avals),
                in_names=tuple(all_in),
                out_names=tuple(out_names),
                lowering_input_output_aliases=(),
                sim_require_finite=True,
                sim_require_nnan=True,
                nc=nc,
            )
            return tuple(outs)

        devices = jax.devices()[:NCORES]
        self.mesh = Mesh(np.asarray(devices), ("core",))
        self._P = PartitionSpec
        self._NS = NamedSharding
        in_specs = tuple(
            PartitionSpec("core") if n in _PER_CORE else PartitionSpec()
            for n in in_names) + (PartitionSpec("core"),) * n_outs
        out_specs = (PartitionSpec("core"),) * n_outs
        self.fn = jax.jit(
            shard_map(_body, mesh=self.mesh, in_specs=in_specs,
                      out_specs=out_specs, check_rep=False),
            donate_argnums=donate, keep_unused=True)
        self._weight_cache = {}

    def put_shared(self, name, arr):
        key = (name, id(arr))
        if key not in self._weight_cache:
            self._weight_cache.clear() if len(self._weight_cache) > 64 else None
            self._weight_cache[key] = self._jax.device_put(
                arr, self._NS(self.mesh, self._P()))
        return self._weight_cache[key]

    def __call__(self, shared_map, per_core_maps):
        args = []
        for n in self.in_names:
            if n in _PER_CORE:
                args.append(np.concatenate(
                    [per_core_maps[c][n] for c in range(NCORES)], axis=0))
            else:
                args.append(self.put_shared(n, shared_map[n]))
        zeros = [np.zeros((NCORES * s[0],) + tuple(s[1:]), dt)
                 for s, dt in self.zero_shapes]
        outs = self.fn(*args, *zeros)
        res = []
        for c in range(NCORES):
            res.append({
                name: np.asarray(outs[i]).reshape(
                    (NCORES,) + self.out_avals[i].shape)[c]
                for i, name in enumerate(self.out_names)})
        return res


@functools.lru_cache(maxsize=2)
def _get_runner(n_layers=L, w_bf16=W_BF16, a_bf16=A_BF16, stage=99, asub=99, dbg=False, nr=NCORES):
    nc = _build(n_layers, w_bf16, a_bf16, stage, asub, dbg, nr)
    return _Runner(nc)


_SHARED_CACHE = {}


def run_device(inputs, n_layers=L, w_bf16=W_BF16, a_bf16=A_BF16):
    """Run the device part; returns per-core result dicts."""
    runner = _get_runner(n_layers, w_bf16, a_bf16)
    key = (id(inputs.get("qkv_w")), n_layers, w_bf16)
    if key not in _SHARED_CACHE:
        if len(_SHARED_CACHE) > 4:
            _SHARED_CACHE.clear()
        _SHARED_CACHE[key] = _prep_shared(inputs, n_layers, w_bf16)
    shared = _SHARED_CACHE[key]
    per_core = _prep_per_core(inputs)
    return runner(shared, per_core)


def kernel(**inputs):
    results = run_device(inputs)
    return _postprocess(results, inputs)
